# revision 35
# baseline (speedup 1.0000x reference)
"""Multi-head causal attention (kqv proj + softmax(QK^T)V) on 8 TRN2 NeuronCores.

Sharding: 8 cores = 4 batches x 2 head-groups (8 heads each). Each core is
fully independent (no collectives); host shards inputs / concats outputs.

Per-core kernel (bf16 matmuls, f32 psum/output):
  phase 1: Q^T/K^T [64, T] per head (2 heads packed into 128 partitions) and
           V [T, 64] per head produced straight from the kqv matmul -- layouts
           chosen so no on-device transpose is ever needed.
  phase 2: S^T[j,i] = K^T.T @ Q^T tiles (only causal j<=i tiles) written
           into 2-bank PSUM groups so one ScalarE exp covers 2 tiles
           (amortizes the ~350-cycle ACT overhead; ScalarE exp is the
           bottleneck engine), 1/8 scale fused into the exp, 0/1 mask
           multiply only on the 128x128 diagonal boundary blocks, then
           out[i,:] = (E^T.T @ [V|1]) normalized by the appended
           denominator column + v-bias.
Head pairs are processed together with even/odd S matmuls interleaved: their
lhsT live at base partitions 0/64, so the PE runs them concurrently via
row-group tiling.
"""

import sys

if "/opt/trn_rl_repo" not in sys.path:
    sys.path.insert(0, "/opt/trn_rl_repo")

import numpy as np
import ml_dtypes

DIM = 1024
NUM_HEADS = 16
SEQ = 2048
BATCH = 4
D = 64  # head dim
SCALE = D**-0.5
N_CORES = 8
HPC = 8  # heads per core
PAIRS = HPC // 2
CC = DIM // 128  # contraction chunks (8)
TCH = SEQ // 512  # 512-wide token chunks (4)
TT = SEQ // 128  # 128-wide token tiles (16)
SG = 2  # j-tiles per S psum group (2 banks); 4ic+4 is even so groups are exact

BF16 = ml_dtypes.bfloat16

_CACHE = {}


def _build_nc():
    import concourse.tile as tile
    from concourse import bacc, mybir

    bf = mybir.dt.bfloat16
    f32 = mybir.dt.float32
    mult = mybir.AluOpType.mult
    add = mybir.AluOpType.add

    nc = bacc.Bacc("TRN2", target_bir_lowering=False)

    xT_d = nc.declare_dram_parameter("xT", [DIM, SEQ], bf, isOutput=False)
    wT_d = nc.declare_dram_parameter("wT", [DIM, 3 * 512], bf, isOutput=False)
    bqk_d = nc.declare_dram_parameter("bqk", [128, 2 * PAIRS], f32, isOutput=False)
    bv_d = nc.declare_dram_parameter("bv", [128, 512], f32, isOutput=False)
    out_d = nc.declare_dram_parameter("out", [SEQ, 512], f32, isOutput=True)

    with tile.TileContext(nc) as tc:
        with (
            tc.tile_pool(name="persist", bufs=1) as persist,
            tc.tile_pool(name="epool", bufs=16) as epool,
            tc.tile_pool(name="ost", bufs=1) as ost,
            tc.tile_pool(name="rpool", bufs=8) as rpool,
            tc.tile_pool(name="psg", bufs=3, space="PSUM") as psg,
            tc.tile_pool(name="ppv", bufs=2, space="PSUM") as ppv,
        ):
            # ---- constants ----
            zb = persist.tile([128, 1], f32, tag="zb")
            nc.vector.memset(zb, 0.0)

            bqk_sb = persist.tile([128, 2 * PAIRS], f32, tag="bqk")
            nc.sync.dma_start(out=bqk_sb, in_=bqk_d[:])
            bv_sb = persist.tile([128, 512], f32, tag="bv")
            nc.sync.dma_start(out=bv_sb, in_=bv_d[:])

            # single causal boundary mask: mask[jj, ii] = 1 if ii >= jj
            mask = persist.tile([128, 128], bf, tag="mask")
            nc.gpsimd.memset(mask, 1.0)
            nc.gpsimd.affine_select(
                out=mask,
                in_=mask,
                compare_op=mybir.AluOpType.is_ge,
                fill=0.0,
                base=0,
                pattern=[[1, 128]],
                channel_multiplier=-1,
            )

            # ---- inputs: xT + qk weight cols first (first-exp critical
            # path), v weight cols after ----
            xT = []
            wT = []
            for c in range(CC):
                t = persist.tile([128, SEQ], bf, tag=f"xT{c}", name=f"xt_in{c}")
                nc.sync.dma_start(out=t, in_=xT_d[c * 128 : (c + 1) * 128, :])
                xT.append(t)
                w = persist.tile([128, 3 * 512], bf, tag=f"wT{c}", name=f"wt_in{c}")
                nc.sync.dma_start(
                    out=w[:, 0:1024], in_=wT_d[c * 128 : (c + 1) * 128, 0:1024]
                )
                wT.append(w)
            for c in range(CC):
                nc.sync.dma_start(
                    out=wT[c][:, 1024:1536],
                    in_=wT_d[c * 128 : (c + 1) * 128, 1024:1536],
                )

            QT = [persist.tile([128, SEQ], bf, tag=f"qt{p}", name=f"qt{p}") for p in range(PAIRS)]
            KT = [persist.tile([128, SEQ], bf, tag=f"kt{p}", name=f"kt{p}") for p in range(PAIRS)]
            Vp = [
                persist.tile([128, HPC, D + 1], bf, tag=f"vp{t}", name=f"vp{t}")
                for t in range(TT)
            ]

            def proj_qk_chunk(p, which, t):
                dst = QT[p] if which == 0 else KT[p]
                bcol = p if which == 0 else PAIRS + p
                wcol = which * 512 + p * 128
                ps = psg.tile([128, 1, 512], f32, tag="sg", name=f"pqk{p}_{which}_{t}")
                for c in range(CC):
                    nc.tensor.matmul(
                        ps[:, 0, :],
                        wT[c][:, wcol : wcol + 128],
                        xT[c][:, t * 512 : (t + 1) * 512],
                        start=(c == 0),
                        stop=(c == CC - 1),
                    )
                nc.vector.tensor_scalar_add(
                    dst[:, t * 512 : (t + 1) * 512],
                    ps[:, 0, :],
                    bqk_sb[:, bcol : bcol + 1],
                )

            def proj_v(tt):
                ps = psg.tile([128, 1, 512], f32, tag="sg", name=f"pv{tt}")
                for c in range(CC):
                    nc.tensor.matmul(
                        ps[:, 0, :],
                        xT[c][:, tt * 128 : (tt + 1) * 128],
                        wT[c][:, 1024:1536],
                        start=(c == 0),
                        stop=(c == CC - 1),
                    )
                nc.vector.tensor_copy(
                    out=Vp[tt][:, :, 0:D],
                    in_=ps[:, 0, :].rearrange("p (h d) -> p h d", h=HPC),
                )
                nc.vector.memset(Vp[tt][:, :, D : D + 1], 1.0)

            # ---- attention, software-pipelined at S-group granularity ----
            # Unit = (pr, ic, g0): SG j-tiles x 2 heads. Emission order is
            # S+exp(unit k+1) then PV(unit k), so the PE computes the next
            # group's scores while ScalarE (the bottleneck) exps, and PV work
            # never sits between an exp and the S matmuls it needs. Projection
            # chains queued by first-need key are sprinkled one per unit.
            pvs = {}

            def emit_s_exp(pr, ic, g0):
                jts = list(range(g0, g0 + SG))
                ps = {}
                for half in (0, 1):
                    ps[half] = psg.tile(
                        [128, SG, 512], f32, tag="sg", name=f"s_{pr}_{ic}_{g0}_{half}"
                    )
                for k, jt in enumerate(jts):
                    for half in (0, 1):  # explicit row groups -> concurrent MMs
                        po = half * D
                        nc.tensor.matmul(
                            ps[half][:, k, :],
                            KT[pr][po : po + D, jt * 128 : (jt + 1) * 128],
                            QT[pr][po : po + D, ic * 512 : (ic + 1) * 512],
                            tile_position=(po, 0),
                        )
                es = {}
                for half in (0, 1):
                    e = epool.tile([128, SG, 512], bf, tag="e")
                    if g0 - 4 * ic == 2:
                        # upper diagonal group: cols < 256 of both banks are
                        # entirely above the causal boundary and never read
                        exp_dst, exp_src = e[:, :, 256:], ps[half][:, :, 256:]
                    else:
                        exp_dst = e[:].rearrange("p a b -> p (a b)")
                        exp_src = ps[half][:].rearrange("p a b -> p (a b)")
                    nc.scalar.activation(
                        exp_dst,
                        exp_src,
                        mybir.ActivationFunctionType.Exp,
                        bias=zb,
                        scale=SCALE,
                    )
                    for k, jt in enumerate(jts):
                        r = jt - 4 * ic
                        if r >= 0:  # diagonal tile: mask the boundary block
                            nc.vector.tensor_tensor(
                                e[:, k, r * 128 : (r + 1) * 128],
                                e[:, k, r * 128 : (r + 1) * 128],
                                mask,
                                mult,
                            )
                    es[half] = e
                return (pr, ic, g0, es)

            def emit_pv(unit):
                pr, ic, g0, es = unit
                if g0 == 0:
                    for half in (0, 1):
                        pvs[half] = ppv.tile(
                            [128, 4, D + 1], f32, tag="pv", name=f"pv_{pr}_{ic}_{half}"
                        )
                # has_written is cleared bank-wide by start=True, so only the
                # first matmul into the pv tile may carry start=True;
                # start=False matmuls overwrite where the bit is unset, which
                # correctly begins the other three chains.
                for half in (0, 1):
                    h = 2 * pr + half
                    e = es[half]
                    for k, jt in enumerate(range(g0, g0 + SG)):
                        for itl in range(max(0, jt - 4 * ic), 4):
                            nc.tensor.matmul(
                                pvs[half][:, itl, :],
                                e[:, k, itl * 128 : (itl + 1) * 128],
                                Vp[jt][:, h, :],
                                start=(jt == 0 and itl == 0),
                                stop=(jt == 4 * ic + 3 and itl == 3),
                            )
                if g0 + SG == 4 * ic + 4:  # last group of the chunk: epilogue
                    stage = stages[ic]
                    for half in (0, 1):
                        h = 2 * pr + half
                        rec = rpool.tile([128, 4], f32, tag="rec")
                        nc.vector.reciprocal(rec, pvs[half][:, :, D])
                        seg = stage[:, :, h * D : (h + 1) * D]
                        nc.vector.tensor_tensor(
                            seg,
                            pvs[half][:, :, 0:D],
                            rec[:, :, None].to_broadcast([128, 4, D]),
                            mult,
                        )
                        nc.vector.tensor_tensor(
                            seg,
                            seg,
                            bv_sb[:, None, h * D : (h + 1) * D].to_broadcast([128, 4, D]),
                            add,
                        )
                    # stream this pair's 128-col output segment immediately
                    nc.sync.dma_start(
                        out=out_d[
                            ic * 512 : (ic + 1) * 512, pr * 128 : (pr + 1) * 128
                        ].rearrange("(a p) c -> p a c", p=128),
                        in_=stage[:, :, pr * 128 : (pr + 1) * 128],
                    )

            stages = [
                ost.tile([128, 4, 512], f32, tag=f"ostage{ic}", name=f"stage_{ic}")
                for ic in range(TCH)
            ]

            # Chunks run in wavefront order -- (0,0),(0,1),(1,0),(0,2),... --
            # so ACT-heavy later-pair chunks overlap the projection-heavy
            # early window. Projection chains are due at the unit index that
            # first needs them: mandatory drain at their due unit, plus one
            # lookahead pop per unit to smooth PE load.
            chunks = sorted(
                [(pr, ic) for pr in range(PAIRS) for ic in range(TCH)],
                key=lambda c: (c[0] + c[1], -c[0]),
            )
            units = [
                (pr, ic, g0) for pr, ic in chunks for g0 in range(0, 4 * ic + 4, SG)
            ]
            uidx = {u: i for i, u in enumerate(units)}

            queue = []
            for pr, ic in chunks:
                due = uidx[(pr, ic, 0)]
                queue.append((due, lambda pr=pr, ic=ic: proj_qk_chunk(pr, 1, ic)))
                queue.append((due, lambda pr=pr, ic=ic: proj_qk_chunk(pr, 0, ic)))
                if pr == 0:
                    for t in range(4 * ic, 4 * ic + 4):
                        queue.append(
                            (uidx[(0, ic, (t // SG) * SG)], lambda t=t: proj_v(t))
                        )
            queue.sort(key=lambda kv: kv[0])

            qi = [0]

            def drain_due(i, lookahead=0, limit=None):
                n = 0
                while qi[0] < len(queue) and queue[qi[0]][0] <= i + lookahead:
                    if lookahead and limit is not None and n >= limit:
                        break
                    queue[qi[0]][1]()
                    qi[0] += 1
                    n += 1

            prev = None
            for i, (pr, ic, g0) in enumerate(units):
                drain_due(i)
                st = emit_s_exp(pr, ic, g0)
                if prev is not None:
                    emit_pv(prev)
                    drain_due(i, lookahead=12, limit=1)
                prev = st
            emit_pv(prev)
            drain_due(len(units))

    nc.compile()
    return nc


def _get_nc():
    if "nc" not in _CACHE:
        _CACHE["nc"] = _build_nc()
    return _CACHE["nc"]


def _make_in_maps(x, w_kqv, b_kqv):
    """Shard: core c -> batch c//2, head-group c%2 (heads hg*8..hg*8+7)."""
    in_maps = []
    for c in range(N_CORES):
        b, hg = divmod(c, 2)
        h0 = hg * HPC
        xT = np.ascontiguousarray(x[b].T).astype(BF16)

        # weight columns, all transposed to [DIM(c), out]:
        # [q pairs (4x128) | k pairs (4x128) | v heads (512)]
        cols = []
        for which in (1, 0):  # q rows live at 1024+, k rows at 0+
            base = which * DIM
            for p in range(PAIRS):
                rows = w_kqv[base + (h0 + 2 * p) * D : base + (h0 + 2 * p + 2) * D, :]
                cols.append(rows.T)
        cols.append(w_kqv[2 * DIM + h0 * D : 2 * DIM + (h0 + HPC) * D, :].T)
        wT = np.ascontiguousarray(np.concatenate(cols, axis=1)).astype(BF16)

        bqk = np.empty((128, 2 * PAIRS), np.float32)
        for p in range(PAIRS):
            bqk[:, p] = b_kqv[DIM + (h0 + 2 * p) * D : DIM + (h0 + 2 * p + 2) * D]
            bqk[:, PAIRS + p] = b_kqv[(h0 + 2 * p) * D : (h0 + 2 * p + 2) * D]
        bv = np.tile(
            b_kqv[2 * DIM + h0 * D : 2 * DIM + (h0 + HPC) * D][None, :].astype(
                np.float32
            ),
            (128, 1),
        )
        in_maps.append({"xT": xT, "wT": wT, "bqk": bqk, "bv": bv})
    return in_maps


def run(x, w_kqv, b_kqv, trace=False, **kwargs):
    from concourse.bass_utils import run_bass_kernel_spmd

    nc = _get_nc()
    in_maps = _make_in_maps(x, w_kqv, b_kqv)
    res = run_bass_kernel_spmd(
        nc, in_maps, core_ids=list(range(N_CORES)), trace=trace, **kwargs
    )
    out = np.empty((BATCH, SEQ, DIM), np.float32)
    for c in range(N_CORES):
        b, hg = divmod(c, 2)
        out[b, :, hg * 512 : (hg + 1) * 512] = res.results[c]["out"]
    return out, res


def kernel(x, w_kqv, b_kqv):
    args = (
        np.asarray(x, np.float32),
        np.asarray(w_kqv, np.float32),
        np.asarray(b_kqv, np.float32),
    )
    try:
        out, _ = run(*args)
    except Exception:
        # transient NRT/device errors recover on retry
        out, _ = run(*args)
    return out


# revision 36
# speedup vs baseline: 1.0362x; 1.0362x over previous
"""Multi-head causal attention (kqv proj + softmax(QK^T)V) on 8 TRN2 NeuronCores.

Sharding: 8 cores = 4 batches x 2 head-groups (8 heads each). Each core is
fully independent (no collectives); host shards inputs / concats outputs.

Per-core kernel (bf16 matmuls, f32 psum/output):
  phase 1: Q^T/K^T [64, T] per head (2 heads packed into 128 partitions) and
           V [T, 64] per head produced straight from the kqv matmul -- layouts
           chosen so no on-device transpose is ever needed.
  phase 2: S^T[j,i] = K^T.T @ Q^T tiles (only causal j<=i tiles) written
           into 2-bank PSUM groups so one ScalarE exp covers 2 tiles
           (amortizes the ~350-cycle ACT overhead; ScalarE exp is the
           bottleneck engine), 1/8 scale fused into the exp, 0/1 mask
           multiply only on the 128x128 diagonal boundary blocks, then
           out[i,:] = (E^T.T @ [V|1]) normalized by the appended
           denominator column + v-bias.
Head pairs are processed together with even/odd S matmuls interleaved: their
lhsT live at base partitions 0/64, so the PE runs them concurrently via
row-group tiling.
"""

import sys

if "/opt/trn_rl_repo" not in sys.path:
    sys.path.insert(0, "/opt/trn_rl_repo")

import numpy as np
import ml_dtypes

DIM = 1024
NUM_HEADS = 16
SEQ = 2048
BATCH = 4
D = 64  # head dim
SCALE = D**-0.5
N_CORES = 8
HPC = 8  # heads per core
PAIRS = HPC // 2
CC = DIM // 128  # contraction chunks (8)
TCH = SEQ // 512  # 512-wide token chunks (4)
TT = SEQ // 128  # 128-wide token tiles (16)
SG = 2  # j-tiles per S psum group (2 banks); 4ic+4 is even so groups are exact

BF16 = ml_dtypes.bfloat16

_CACHE = {}


def _build_nc():
    import concourse.tile as tile
    from concourse import bacc, mybir

    bf = mybir.dt.bfloat16
    f32 = mybir.dt.float32
    mult = mybir.AluOpType.mult
    add = mybir.AluOpType.add

    nc = bacc.Bacc("TRN2", target_bir_lowering=False)

    xT_d = nc.declare_dram_parameter("xT", [DIM, SEQ], bf, isOutput=False)
    wT_d = nc.declare_dram_parameter("wT", [DIM, 3 * 512], bf, isOutput=False)
    bqk_d = nc.declare_dram_parameter("bqk", [128, 2 * PAIRS], f32, isOutput=False)
    bv_d = nc.declare_dram_parameter("bv", [128, 512], f32, isOutput=False)
    out_d = nc.declare_dram_parameter("out", [SEQ, 512], f32, isOutput=True)

    with tile.TileContext(nc) as tc:
        with (
            tc.tile_pool(name="persist", bufs=1) as persist,
            tc.tile_pool(name="epool", bufs=16) as epool,
            tc.tile_pool(name="ost", bufs=1) as ost,
            tc.tile_pool(name="rpool", bufs=8) as rpool,
            tc.tile_pool(name="psg", bufs=3, space="PSUM") as psg,
            tc.tile_pool(name="ppv", bufs=2, space="PSUM") as ppv,
        ):
            # ---- constants ----
            zb = persist.tile([128, 1], f32, tag="zb")
            nc.vector.memset(zb, 0.0)

            bqk_sb = persist.tile([128, 2 * PAIRS], f32, tag="bqk")
            nc.sync.dma_start(out=bqk_sb, in_=bqk_d[:])
            bv_sb = persist.tile([128, 512], f32, tag="bv")
            nc.sync.dma_start(out=bv_sb, in_=bv_d[:])

            # single causal boundary mask: mask[jj, ii] = 1 if ii >= jj
            mask = persist.tile([128, 128], bf, tag="mask")
            nc.gpsimd.memset(mask, 1.0)
            nc.gpsimd.affine_select(
                out=mask,
                in_=mask,
                compare_op=mybir.AluOpType.is_ge,
                fill=0.0,
                base=0,
                pattern=[[1, 128]],
                channel_multiplier=-1,
            )

            # ---- inputs, in arrival-priority waves. The first exp only
            # needs xT cols 0:512 + the qk weight cols, so those land first;
            # later token columns and v weights follow. Separate tiles per
            # wave so Tile's RAW tracking doesn't serialize on later DMAs.
            xTa = []  # cols 0:512
            xTb = []  # cols 512:1024
            xTc = []  # cols 1024:2048
            wT = []
            for c in range(CC):
                t = persist.tile([128, 512], bf, tag=f"xTa{c}", name=f"xta{c}")
                nc.sync.dma_start(out=t, in_=xT_d[c * 128 : (c + 1) * 128, 0:512])
                xTa.append(t)
                w = persist.tile([128, 3 * 512], bf, tag=f"wT{c}", name=f"wt_in{c}")
                nc.sync.dma_start(
                    out=w[:, 0:1024], in_=wT_d[c * 128 : (c + 1) * 128, 0:1024]
                )
                wT.append(w)
            for c in range(CC):
                t = persist.tile([128, 512], bf, tag=f"xTb{c}", name=f"xtb{c}")
                nc.sync.dma_start(out=t, in_=xT_d[c * 128 : (c + 1) * 128, 512:1024])
                xTb.append(t)
            for c in range(CC):
                nc.sync.dma_start(
                    out=wT[c][:, 1024:1536],
                    in_=wT_d[c * 128 : (c + 1) * 128, 1024:1536],
                )
            for c in range(CC):
                t = persist.tile([128, 1024], bf, tag=f"xTc{c}", name=f"xtc{c}")
                nc.sync.dma_start(out=t, in_=xT_d[c * 128 : (c + 1) * 128, 1024:2048])
                xTc.append(t)

            def x_cols(c, lo, width):
                """view of xT[c][:, lo:lo+width] from the wave tiles"""
                if lo + width <= 512:
                    return xTa[c][:, lo : lo + width]
                if lo >= 512 and lo + width <= 1024:
                    return xTb[c][:, lo - 512 : lo - 512 + width]
                assert lo >= 1024
                return xTc[c][:, lo - 1024 : lo - 1024 + width]

            QT = [persist.tile([128, SEQ], bf, tag=f"qt{p}", name=f"qt{p}") for p in range(PAIRS)]
            KT = [persist.tile([128, SEQ], bf, tag=f"kt{p}", name=f"kt{p}") for p in range(PAIRS)]
            Vp = [
                persist.tile([128, HPC, D + 1], bf, tag=f"vp{t}", name=f"vp{t}")
                for t in range(TT)
            ]

            def proj_qk_chunk(p, which, t):
                dst = QT[p] if which == 0 else KT[p]
                bcol = p if which == 0 else PAIRS + p
                wcol = which * 512 + p * 128
                ps = psg.tile([128, 1, 512], f32, tag="sg", name=f"pqk{p}_{which}_{t}")
                for c in range(CC):
                    nc.tensor.matmul(
                        ps[:, 0, :],
                        wT[c][:, wcol : wcol + 128],
                        x_cols(c, t * 512, 512),
                        start=(c == 0),
                        stop=(c == CC - 1),
                    )
                nc.vector.tensor_scalar_add(
                    dst[:, t * 512 : (t + 1) * 512],
                    ps[:, 0, :],
                    bqk_sb[:, bcol : bcol + 1],
                )

            def proj_v(tt):
                ps = psg.tile([128, 1, 512], f32, tag="sg", name=f"pv{tt}")
                for c in range(CC):
                    nc.tensor.matmul(
                        ps[:, 0, :],
                        x_cols(c, tt * 128, 128),
                        wT[c][:, 1024:1536],
                        start=(c == 0),
                        stop=(c == CC - 1),
                    )
                nc.vector.tensor_copy(
                    out=Vp[tt][:, :, 0:D],
                    in_=ps[:, 0, :].rearrange("p (h d) -> p h d", h=HPC),
                )
                nc.vector.memset(Vp[tt][:, :, D : D + 1], 1.0)

            # ---- attention, software-pipelined at S-group granularity ----
            # Unit = (pr, ic, g0): SG j-tiles x 2 heads. Emission order is
            # S+exp(unit k+1) then PV(unit k), so the PE computes the next
            # group's scores while ScalarE (the bottleneck) exps, and PV work
            # never sits between an exp and the S matmuls it needs. Projection
            # chains queued by first-need key are sprinkled one per unit.
            pvs = {}

            def emit_s_exp(pr, ic, g0):
                jts = list(range(g0, g0 + SG))
                ps = {}
                for half in (0, 1):
                    ps[half] = psg.tile(
                        [128, SG, 512], f32, tag="sg", name=f"s_{pr}_{ic}_{g0}_{half}"
                    )
                for k, jt in enumerate(jts):
                    for half in (0, 1):  # explicit row groups -> concurrent MMs
                        po = half * D
                        nc.tensor.matmul(
                            ps[half][:, k, :],
                            KT[pr][po : po + D, jt * 128 : (jt + 1) * 128],
                            QT[pr][po : po + D, ic * 512 : (ic + 1) * 512],
                            tile_position=(po, 0),
                        )
                es = {}
                for half in (0, 1):
                    e = epool.tile([128, SG, 512], bf, tag="e")
                    if g0 - 4 * ic == 2:
                        # upper diagonal group: cols < 256 of both banks are
                        # entirely above the causal boundary and never read
                        exp_dst, exp_src = e[:, :, 256:], ps[half][:, :, 256:]
                    else:
                        exp_dst = e[:].rearrange("p a b -> p (a b)")
                        exp_src = ps[half][:].rearrange("p a b -> p (a b)")
                    nc.scalar.activation(
                        exp_dst,
                        exp_src,
                        mybir.ActivationFunctionType.Exp,
                        bias=zb,
                        scale=SCALE,
                    )
                    for k, jt in enumerate(jts):
                        r = jt - 4 * ic
                        if r >= 0:  # diagonal tile: mask the boundary block
                            nc.vector.tensor_tensor(
                                e[:, k, r * 128 : (r + 1) * 128],
                                e[:, k, r * 128 : (r + 1) * 128],
                                mask,
                                mult,
                            )
                    es[half] = e
                return (pr, ic, g0, es)

            def emit_pv(unit):
                pr, ic, g0, es = unit
                if g0 == 0:
                    for half in (0, 1):
                        pvs[half] = ppv.tile(
                            [128, 4, D + 1], f32, tag="pv", name=f"pv_{pr}_{ic}_{half}"
                        )
                # has_written is cleared bank-wide by start=True, so only the
                # first matmul into the pv tile may carry start=True;
                # start=False matmuls overwrite where the bit is unset, which
                # correctly begins the other three chains.
                for half in (0, 1):
                    h = 2 * pr + half
                    e = es[half]
                    for k, jt in enumerate(range(g0, g0 + SG)):
                        for itl in range(max(0, jt - 4 * ic), 4):
                            nc.tensor.matmul(
                                pvs[half][:, itl, :],
                                e[:, k, itl * 128 : (itl + 1) * 128],
                                Vp[jt][:, h, :],
                                start=(jt == 0 and itl == 0),
                                stop=(jt == 4 * ic + 3 and itl == 3),
                            )
                if g0 + SG == 4 * ic + 4:  # last group of the chunk: epilogue
                    stage = stages[ic]
                    for half in (0, 1):
                        h = 2 * pr + half
                        rec = rpool.tile([128, 4], f32, tag="rec")
                        nc.vector.reciprocal(rec, pvs[half][:, :, D])
                        seg = stage[:, :, h * D : (h + 1) * D]
                        nc.vector.tensor_tensor(
                            seg,
                            pvs[half][:, :, 0:D],
                            rec[:, :, None].to_broadcast([128, 4, D]),
                            mult,
                        )
                        nc.vector.tensor_tensor(
                            seg,
                            seg,
                            bv_sb[:, None, h * D : (h + 1) * D].to_broadcast([128, 4, D]),
                            add,
                        )
                    # stream this pair's 128-col output segment immediately
                    nc.sync.dma_start(
                        out=out_d[
                            ic * 512 : (ic + 1) * 512, pr * 128 : (pr + 1) * 128
                        ].rearrange("(a p) c -> p a c", p=128),
                        in_=stage[:, :, pr * 128 : (pr + 1) * 128],
                    )

            stages = [
                ost.tile([128, 4, 512], f32, tag=f"ostage{ic}", name=f"stage_{ic}")
                for ic in range(TCH)
            ]

            # Chunks run in wavefront order -- (0,0),(0,1),(1,0),(0,2),... --
            # so ACT-heavy later-pair chunks overlap the projection-heavy
            # early window. Projection chains are due at the unit index that
            # first needs them: mandatory drain at their due unit, plus one
            # lookahead pop per unit to smooth PE load.
            chunks = sorted(
                [(pr, ic) for pr in range(PAIRS) for ic in range(TCH)],
                key=lambda c: (c[0] + c[1], -c[0]),
            )
            units = [
                (pr, ic, g0) for pr, ic in chunks for g0 in range(0, 4 * ic + 4, SG)
            ]
            uidx = {u: i for i, u in enumerate(units)}

            queue = []
            for pr, ic in chunks:
                due = uidx[(pr, ic, 0)]
                queue.append((due, lambda pr=pr, ic=ic: proj_qk_chunk(pr, 1, ic)))
                queue.append((due, lambda pr=pr, ic=ic: proj_qk_chunk(pr, 0, ic)))
                if pr == 0:
                    for t in range(4 * ic, 4 * ic + 4):
                        queue.append(
                            (uidx[(0, ic, (t // SG) * SG)], lambda t=t: proj_v(t))
                        )
            queue.sort(key=lambda kv: kv[0])

            qi = [0]

            def drain_due(i, lookahead=0, limit=None):
                n = 0
                while qi[0] < len(queue) and queue[qi[0]][0] <= i + lookahead:
                    if lookahead and limit is not None and n >= limit:
                        break
                    queue[qi[0]][1]()
                    qi[0] += 1
                    n += 1

            prev = None
            for i, (pr, ic, g0) in enumerate(units):
                drain_due(i)
                st = emit_s_exp(pr, ic, g0)
                if prev is not None:
                    emit_pv(prev)
                    drain_due(i, lookahead=12, limit=1)
                prev = st
            emit_pv(prev)
            drain_due(len(units))

    nc.compile()
    return nc


def _get_nc():
    if "nc" not in _CACHE:
        _CACHE["nc"] = _build_nc()
    return _CACHE["nc"]


def _make_in_maps(x, w_kqv, b_kqv):
    """Shard: core c -> batch c//2, head-group c%2 (heads hg*8..hg*8+7)."""
    in_maps = []
    for c in range(N_CORES):
        b, hg = divmod(c, 2)
        h0 = hg * HPC
        xT = np.ascontiguousarray(x[b].T).astype(BF16)

        # weight columns, all transposed to [DIM(c), out]:
        # [q pairs (4x128) | k pairs (4x128) | v heads (512)]
        cols = []
        for which in (1, 0):  # q rows live at 1024+, k rows at 0+
            base = which * DIM
            for p in range(PAIRS):
                rows = w_kqv[base + (h0 + 2 * p) * D : base + (h0 + 2 * p + 2) * D, :]
                cols.append(rows.T)
        cols.append(w_kqv[2 * DIM + h0 * D : 2 * DIM + (h0 + HPC) * D, :].T)
        wT = np.ascontiguousarray(np.concatenate(cols, axis=1)).astype(BF16)

        bqk = np.empty((128, 2 * PAIRS), np.float32)
        for p in range(PAIRS):
            bqk[:, p] = b_kqv[DIM + (h0 + 2 * p) * D : DIM + (h0 + 2 * p + 2) * D]
            bqk[:, PAIRS + p] = b_kqv[(h0 + 2 * p) * D : (h0 + 2 * p + 2) * D]
        bv = np.tile(
            b_kqv[2 * DIM + h0 * D : 2 * DIM + (h0 + HPC) * D][None, :].astype(
                np.float32
            ),
            (128, 1),
        )
        in_maps.append({"xT": xT, "wT": wT, "bqk": bqk, "bv": bv})
    return in_maps


def run(x, w_kqv, b_kqv, trace=False, **kwargs):
    from concourse.bass_utils import run_bass_kernel_spmd

    nc = _get_nc()
    in_maps = _make_in_maps(x, w_kqv, b_kqv)
    res = run_bass_kernel_spmd(
        nc, in_maps, core_ids=list(range(N_CORES)), trace=trace, **kwargs
    )
    out = np.empty((BATCH, SEQ, DIM), np.float32)
    for c in range(N_CORES):
        b, hg = divmod(c, 2)
        out[b, :, hg * 512 : (hg + 1) * 512] = res.results[c]["out"]
    return out, res


def kernel(x, w_kqv, b_kqv):
    args = (
        np.asarray(x, np.float32),
        np.asarray(w_kqv, np.float32),
        np.asarray(b_kqv, np.float32),
    )
    try:
        out, _ = run(*args)
    except Exception:
        # transient NRT/device errors recover on retry
        out, _ = run(*args)
    return out
